# revision 13
# baseline (speedup 1.0000x reference)
"""Trainium2 Bass kernel for AdaptiveRevRNN.

Reference computation (per step t over T = 2056 steps, batch B = 128):
    h = [h0 | h1], each [B, 256]
    o1 = h1 + calc(h0, seq_t, lp0);  o0 = h0 + calc(o1, seq_t, lp1)
    calc = 2 stacked layers of:  z = (h - mean_B(h)) @ W_h + seq @ W_s
                                 h' = relu6(z[:, :256]) * tanh(z[:, 256:])

Design: the recurrence is strictly serial (8224 dependent layer evals), so a
single fused on-core pipeline is run redundantly on all 8 cores (SPMD needs an
identical graph per core; only core 0's output is used).

Layout is fully transposed: state hT [hf, b] so that
 - the matmul contraction (hf) lands on partitions with W as the stationary
   operand and hT as the moving operand (zT = W_slice.T @ hT per 128-wide
   n-slice), and the layer output h'T is produced directly in the layout the
   next layer consumes — no transposes anywhere;
 - the batch mean is a free-dim reduction, obtained for free via accum_out on
   the DVE op that produces each layer input;
 - mean subtraction uses prescaled weights: hs = 128*h - sum_B(h) fed to
   W_h/128, one DVE tensor_scalar per k-tile;
 - the positional-encoding columns of W_s reduce to a per-step per-partition
   bias applied inside the activations (host-precomputed table);
 - the x projection (x_t @ W_s) is accumulated into the same PSUM bank by the
   TensorEngine during the previous layer's activation window.
"""

import os

import numpy as np

import concourse.bass as bass
import concourse.tile as tile
from concourse import mybir
from concourse.bass_utils import run_bass_kernel_spmd

B = 128
S = 2048
F = 256
HF = 256
DELAY = 8
T = S + DELAY  # 2056
CHUNK = 16  # steps per x/pos DMA chunk
BODY = 2 * CHUNK  # steps per For_i body (must be even)
SPAD = S + 3 * CHUNK  # padded x length covering prefetch overrun
TPAD = T + 3 * CHUNK

F16 = mybir.dt.float16
F32 = mybir.dt.float32
AOT = mybir.AluOpType
ACTF = mybir.ActivationFunctionType

LAST_EXEC_NS = None
LAST_RESULT = None
_CACHE = {}


def _build_nc(t_total=T):
    nc = bass.Bass()
    x_ext = nc.declare_dram_parameter("xt", [128, SPAD, 2, 128], F16, isOutput=False)
    wh_ext = nc.declare_dram_parameter("wh", [128, 4, 2, 4, 128], F16, isOutput=False)
    wx_ext = nc.declare_dram_parameter("wx", [128, 4, 2, 4, 128], F16, isOutput=False)
    pb_ext = nc.declare_dram_parameter("pb", [128, 4, 4, TPAD], F32, isOutput=False)
    hi_ext = nc.declare_dram_parameter("hi", [128, 2, 2, 128], F32, isOutput=False)
    out_ext = nc.declare_dram_parameter("out", [128, 2, 2, 128], F32, isOutput=True)

    with tile.TileContext(nc) as tc:
        with (
            tc.tile_pool(name="const", bufs=1) as constp,
            tc.tile_pool(name="xs", bufs=1) as xsp,
            tc.tile_pool(name="state", bufs=1) as statep,
            tc.tile_pool(name="work", bufs=3) as workp,
            tc.tile_pool(name="psum", bufs=8, space="PSUM") as psump,
        ):
            wh = constp.tile([128, 4, 2, 4, 128], F16, tag="wh")
            wx = constp.tile([128, 4, 2, 4, 128], F16, tag="wx")
            nc.sync.dma_start(wh[:], wh_ext[:])
            nc.sync.dma_start(wx[:], wx_ext[:])

            # ping-pong recurrent state: h0/h1 transposed [hf(2x128), b], plus
            # their batch sums [128, kt]
            h0 = [statep.tile([128, 2, 128], F32, tag=f"h0_{i}", name=f"h0_{i}") for i in range(2)]
            h1 = [statep.tile([128, 2, 128], F32, tag=f"h1_{i}", name=f"h1_{i}") for i in range(2)]
            s0 = [statep.tile([128, 2], F32, tag=f"s0_{i}", name=f"s0_{i}") for i in range(2)]
            s1 = [statep.tile([128, 2], F32, tag=f"s1_{i}", name=f"s1_{i}") for i in range(2)]

            nc.sync.dma_start(h0[0][:], hi_ext[:, 0])
            nc.sync.dma_start(h1[0][:], hi_ext[:, 1])
            for kt in range(2):
                nc.vector.reduce_sum(
                    s0[0][:, kt : kt + 1], h0[0][:, kt, :], axis=mybir.AxisListType.X
                )
                nc.vector.reduce_sum(
                    s1[0][:, kt : kt + 1], h1[0][:, kt, :], axis=mybir.AxisListType.X
                )

            xc = [
                xsp.tile([128, CHUNK, 2, 128], F16, tag=f"xc_{i}", name=f"xc_{i}") for i in range(2)
            ]
            pbc = [
                xsp.tile([128, 4, 4, CHUNK], F32, tag=f"pbc_{i}", name=f"pbc_{i}") for i in range(2)
            ]
            nc.sync.dma_start(xc[0][:], x_ext[:, 0:CHUNK])
            nc.sync.dma_start(pbc[0][:], pb_ext[:, :, :, 0:CHUNK])

            def do_layer(L, hin, sums, pb_t, tl, xc_t, has_x, out_f16, acc_sums):
                """One single_calc layer: returns (h'_tile, sums_tile or None).

                hin: [128, 2, 128] (f32 or f16), sums: [128, 2] f32.
                """
                m = workp.tile([128, 2], F32, tag="m", name="m")
                nc.vector.tensor_scalar(m[:], sums[:], 1.0 / 128.0, None, AOT.mult)
                hs = workp.tile([128, 2, 128], F16, tag="hs")
                for kt in range(2):
                    nc.vector.tensor_scalar(
                        hs[:, kt, :],
                        hin[:, kt, :],
                        m[:, kt : kt + 1],
                        None,
                        AOT.subtract,
                    )
                z = psump.tile([128, 4, 128], F32, tag="z")
                if has_x:
                    for sl in range(4):
                        for kt in range(2):
                            nc.tensor.matmul(
                                z[:, sl, :],
                                wx[:, L, kt, sl, :],
                                xc_t[:, tl, kt, :],
                                start=(kt == 0),
                                stop=False,
                            )
                for sl in range(4):
                    for kt in range(2):
                        nc.tensor.matmul(
                            z[:, sl, :],
                            wh[:, L, kt, sl, :],
                            hs[:, kt, :],
                            start=(kt == 0 and not has_x),
                            stop=(kt == 1),
                        )
                rm = workp.tile([128, 2, 128], F16, tag="rm")
                th = workp.tile([128, 2, 128], F16, tag="th")
                # all four PSUM reads on ACT: a single release semaphore per
                # bank keeps the wait count on the next writer's LDW low
                nc.scalar.activation(
                    rm[:, 0, :], z[:, 0, :], ACTF.Relu, bias=pb_t[:, L, 0, tl : tl + 1]
                )
                nc.scalar.activation(
                    rm[:, 1, :], z[:, 1, :], ACTF.Relu, bias=pb_t[:, L, 1, tl : tl + 1]
                )
                nc.scalar.activation(
                    th[:, 0, :], z[:, 2, :], ACTF.Tanh, bias=pb_t[:, L, 2, tl : tl + 1]
                )
                nc.scalar.activation(
                    th[:, 1, :], z[:, 3, :], ACTF.Tanh, bias=pb_t[:, L, 3, tl : tl + 1]
                )
                hout = workp.tile([128, 2, 128], F16 if out_f16 else F32, tag="ho")
                souts = workp.tile([128, 2], F32, tag="so", name="so") if acc_sums else None
                for kt in range(2):
                    nc.vector.scalar_tensor_tensor(
                        hout[:, kt, :],
                        rm[:, kt, :],
                        6.0,
                        th[:, kt, :],
                        AOT.min,
                        AOT.mult,
                        accum_out=souts[:, kt : kt + 1] if acc_sums else None,
                    )
                return hout, souts

            def do_step(par, xc_t, pb_t, tl, has_x):
                """One full recurrent step; reads state slot par, writes 1-par."""
                q = 1 - par
                # calc(h0, lp0): layers 0, 1
                ha, sa = do_layer(0, h0[par], s0[par], pb_t, tl, xc_t, has_x, True, True)
                c0, _ = do_layer(1, ha, sa, pb_t, tl, xc_t, has_x, True, False)
                # o1 = h1 + c0, with batch sums for the next layer
                for kt in range(2):
                    nc.vector.scalar_tensor_tensor(
                        h1[q][:, kt, :],
                        c0[:, kt, :],
                        0.0,
                        h1[par][:, kt, :],
                        AOT.bypass,
                        AOT.add,
                        accum_out=s1[q][:, kt : kt + 1],
                    )
                # calc(o1, lp1): layers 2, 3
                hb, sb = do_layer(2, h1[q], s1[q], pb_t, tl, xc_t, has_x, True, True)
                c1, _ = do_layer(3, hb, sb, pb_t, tl, xc_t, has_x, True, False)
                # o0 = h0 + c1
                for kt in range(2):
                    nc.vector.scalar_tensor_tensor(
                        h0[q][:, kt, :],
                        c1[:, kt, :],
                        0.0,
                        h0[par][:, kt, :],
                        AOT.bypass,
                        AOT.add,
                        accum_out=s0[q][:, kt : kt + 1],
                    )

            # fully static unrolled schedule: chunk c covers steps
            # [16c, 16c+16) of the T=2056 total; last chunk is the 8-step
            # zero-x tail. Straight-line code — no loop back-edges, no
            # stage machinery, waits stay sparse.
            nchunk = (t_total + CHUNK - 1) // CHUNK
            for c in range(nchunk):
                if c + 1 < nchunk:
                    t0n = (c + 1) * CHUNK
                    if t0n < S:
                        nc.sync.dma_start(
                            xc[(c + 1) % 2][:], x_ext[:, t0n : t0n + CHUNK]
                        )
                    nc.sync.dma_start(
                        pbc[(c + 1) % 2][:], pb_ext[:, :, :, t0n : t0n + CHUNK]
                    )
                for tl in range(min(CHUNK, t_total - c * CHUNK)):
                    t = c * CHUNK + tl
                    do_step(t % 2, xc[c % 2], pbc[c % 2], tl, t < S)

            assert t_total % 2 == 0
            # even step count -> final state is in slot 0
            nc.sync.dma_start(out_ext[:, 0], h0[0][:])
            nc.sync.dma_start(out_ext[:, 1], h1[0][:])

    return nc


def _fix_ldw_waits(nc):
    """Hardware instructions carry one sync-wait slot; walrus packs a few
    more via auto-inserted NOPs but has per-struct limits (LDWEIGHTS: 1;
    others ~8-10). Two fixups:
      - LDWEIGHTS with >1 wait: keep the weights-arrival (DMA) wait, move
        the rest onto the paired Matmult that follows it in-order.
      - any instruction with >MAXW waits: hoist the excess onto inserted
        same-engine NoOps (1 wait each) right before it — same semantics,
        waits just spread over several CTRL instructions."""
    MAXW = 1
    nopid = [0]
    for blk in nc.m.functions[0].blocks:
        insts = blk.instructions
        for i, inst in enumerate(insts):
            if (
                isinstance(inst, mybir.InstLdweights)
                and inst.sync_info
                and len(inst.sync_info.on_wait) > 1
            ):
                w = list(inst.sync_info.on_wait)
                nxt = insts[i + 1]
                assert isinstance(nxt, mybir.InstMatmult), type(nxt).__name__
                keep = [x for x in w if x.ant_name.startswith("DMA")][:1] or w[:1]
                extra = [x for x in w if x not in keep]
                inst.sync_info = mybir.SyncInfo(
                    on_wait=keep, on_update=list(inst.sync_info.on_update)
                )
                ns = nxt.sync_info
                nxt.sync_info = mybir.SyncInfo(
                    on_wait=(list(ns.on_wait) if ns else []) + extra,
                    on_update=(list(ns.on_update) if ns else []),
                )
        # second sweep: spread any remaining fat wait lists over NoOps
        i = 0
        while i < len(insts):
            inst = insts[i]
            si = inst.sync_info
            if si and len(si.on_wait) > MAXW:
                w = list(si.on_wait)
                keep, extra = w[:MAXW], w[MAXW:]
                inst.sync_info = mybir.SyncInfo(
                    on_wait=keep, on_update=list(si.on_update)
                )
                for j, ew in enumerate(extra):
                    nop = mybir.InstNoOp(
                        name=f"waitnop-{nopid[0]}", ins=[], outs=[]
                    )
                    nopid[0] += 1
                    nop.engine = inst.engine
                    nop.sync_info = mybir.SyncInfo(on_wait=[ew], on_update=[])
                    insts.insert(i, nop)
                    i += 1
            i += 1


def _prep_inputs(x, hidden_state, lp0, lp1):
    lw = np.concatenate([lp0, lp1], axis=0).astype(np.float32)  # [4, 514, 512]

    # stationary weights [k, L, kt, s, m]
    whp = lw[:, :HF, :].reshape(4, 2, 128, 4, 128).transpose(2, 0, 1, 3, 4)
    wh_host = np.ascontiguousarray(whp.astype(np.float16))
    wxp = lw[:, HF : HF + F, :].reshape(4, 2, 128, 4, 128).transpose(2, 0, 1, 3, 4)
    wx_host = np.ascontiguousarray(wxp.astype(np.float16))

    # positional bias table pb[p, L, s, t] = pos[t] @ W_pos[L][:, s*128+p]
    pw = lw[:, HF + F :, :]  # [4, 2, 512]
    idx = np.arange(1, T + 1, dtype=np.float32)
    factor = np.float32((T + 1) / 2.0)
    pos = np.stack([idx, (idx - factor) / factor], axis=-1)  # [T, 2]
    pb = np.einsum("tc,lcn->lnt", pos, pw).astype(np.float32)  # [4, 512, T]
    pb_host = np.zeros((128, 4, 4, TPAD), np.float32)
    pb_host[:, :, :, :T] = pb.reshape(4, 4, 128, T).transpose(2, 0, 1, 3)
    pb_host = np.ascontiguousarray(pb_host)

    # x transposed [k, t, kt, b] fp16, zero-padded in t
    x_host = np.zeros((128, SPAD, 2, 128), np.float16)
    x_host[:, :S] = (
        x.astype(np.float16).transpose(2, 1, 0).reshape(2, 128, S, 128)
    ).transpose(1, 2, 0, 3)
    x_host = np.ascontiguousarray(x_host)

    hi_host = np.ascontiguousarray(
        np.broadcast_to(
            hidden_state.astype(np.float32).reshape(2, 2, 128, 1).transpose(2, 0, 1, 3),
            (128, 2, 2, 128),
        )
    )
    return {
        "xt": x_host,
        "wh": wh_host,
        "wx": wx_host,
        "pb": pb_host,
        "hi": hi_host,
    }


def _install_trace_hook():
    """The container's antenv lacks axon_hooks; recreate the shim so
    run_bass_kernel_spmd's trace path (NTFF profiling over axon) works."""
    import sys as _sys
    import types as _types

    if "antenv.axon_hooks" in _sys.modules:
        return
    try:
        mod = _types.ModuleType("antenv.axon_hooks")
        holder = [None]
        mod.set_axon_ntff_profile_hook = lambda h: holder.__setitem__(0, h)
        mod.get_axon_ntff_profile_hook = lambda: holder[0]
        _sys.modules["antenv.axon_hooks"] = mod
        import antenv

        antenv.axon_hooks = mod
        from trn_agent_boot.trn_boot import _ntff_profile_via_ctypes

        mod.set_axon_ntff_profile_hook(
            _ntff_profile_via_ctypes("/opt/axon/libaxon_pjrt.so")
        )
    except Exception as e:  # degrade to no tracing
        print(f"trace hook install failed: {e}")


def _run(in_map, t_total=T, trace=False):
    global LAST_EXEC_NS, LAST_RESULT
    key = f"nc{t_total}"
    if key not in _CACHE:
        nc = _build_nc(t_total)
        _fix_ldw_waits(nc)
        _CACHE[key] = nc
    nc = _CACHE[key]
    if trace:
        _install_trace_hook()
    res = run_bass_kernel_spmd(
        nc, [in_map for _ in range(8)], core_ids=list(range(8)), trace=trace
    )
    LAST_EXEC_NS = res.exec_time_ns
    LAST_RESULT = res
    out_np = np.asarray(res.results[0]["out"])  # [p, st, kt, b]
    h = out_np.transpose(1, 2, 0, 3).reshape(512, 128).T
    return np.ascontiguousarray(h.astype(np.float32))


def kernel(x, hidden_state, lp0, lp1):
    in_map = _prep_inputs(x, hidden_state, lp0, lp1)
    trace = bool(os.environ.get("KERNEL_TRACE"))
    return _run(in_map, T, trace)
